# revision 61
# baseline (speedup 1.0000x reference)
"""DequantingLinear Trainium2 kernel.

y = x @ W^T + b where W = (w_q - 128) * w_scales (GGML Q8_0-style, block=32),
b = (b_q - 128) * b_scales.

Sharding: column-parallel over out_features across 8 cores (1536 rows of W
per core).  Each core:
  1. DMAs its w_q shard [1536, 3072] int32 tile-by-tile (contiguous, HBM-bound)
  2. Dequantizes on DVE with one fused scalar_tensor_tensor:
         wp = (w_q - 128) * scales   (scales broadcast along free dim, step-0 AP)
  3. Transposes wp 128x128 tiles on the PE (is_transpose matmul) -> PSUM,
     evacuates to SBUF on ACT  (the matmul needs W^T: contraction dim must be
     on partitions for both operands)
  4. Accumulates y[64, 512] = sum_k xT_k^T @ wpT_k in PSUM (24 fp32 matmuls,
     N=512) + one K=1 matmul against ones to add the (device-dequantized) bias
  5. DMAs y shard [64, 1536] out; host concatenates shards.

x is transposed on the host (it is the tiny replicated activation, 768 KB);
all heavy tensors (w_q, w_scales) stream through the device untouched.
"""

import sys

import numpy as np

for _p in ("/opt/trn_rl_repo", "/root/.axon_site/_ro/trn_rl_repo"):
    if _p not in sys.path:
        sys.path.append(_p)

B = 64          # batch (x is [64, 1, 3072])
IN = 3072       # in_features
OUT = 12288     # out_features
BLOCK = 32      # quant block
NB = IN // BLOCK            # 96 blocks per row
NCORES = 8
OSH = OUT // NCORES         # 1536 out features per core
OT = OSH // 128             # 12 o-tiles of 128 rows per core
GRP = 4                     # o-tiles per matmul group (N = 512)
NG = OT // GRP              # 3 groups
KT = IN // 128              # 24 contraction tiles

_CACHE: dict = {}

# PE dtype modes (empirically tuned; see notes):
#  - fp32 matmul = 2 half-speed instruction pairs (4 cycles/row)
#  - float32r = single-pass (1 cycle/row at N>=256); transposes 1.5 vs 2.0
F32R_TR = False  # float32r needs producers rounded (bf16-class precision)
F32R_MM = False
TRMODE_MM = False  # is_transpose with a non-identity rhs does NOT matmul (NaN)
# Half-precision W/x matmul path: dequant stays fp32 on DVE, W is rounded once
# to fp16; PE transposes and matmuls run at 1 cycle/row (vs 4 for fp32) with
# fp32 PSUM accumulation.  fp32 matmul on TRN2 costs 2 half-rate instruction
# passes, so the ~56us/core memory roofline is only reachable this way.
# fp16 (11-bit mantissa) gives ~2.4e-4 relative error vs bf16's ~2e-3 at the
# same PE speed; the value ranges (|W|<=2.6, |x|<6) are far from fp16 limits
# and accumulation is fp32 in PSUM.
HALF = True


def _patch_drain_split():
    """The TRN2 ISA gives every instruction exactly ONE inline wait slot;
    Tile's kernel-tail drain asks for the whole global clock (~11 sems) on a
    single instruction, which walrus sometimes refuses ("Too many sync wait
    commands").  Pre-spread those waits across one SP nop per semaphore; the
    drain's own waits then elide via the SP engine clock."""
    from concourse import tile as tile_mod

    if getattr(tile_mod.TileContext, "_drain_split_patched", False):
        return
    from concourse.vector_clock import ScopedClock, VectorClock

    orig = tile_mod.TileContext._drain_and_barrier

    def patched(self, tick_clock, wait_clock):
        gvc = tick_clock.global_clock
        n = len(gvc)
        for p in range(n):
            t = gvc[p]
            if t <= 0:
                continue
            vc = VectorClock([0] * n)
            vc.require_at_least(p, t)
            nop = self.nc.sync.nop(hint="drain_wait_split", nofuse=True)
            wait_clock.add_sem_waits(nop.ins, ScopedClock({None: vc}))
        return orig(self, tick_clock, wait_clock)

    tile_mod.TileContext._drain_and_barrier = patched
    tile_mod.TileContext._drain_split_patched = True


def _build_nc():
    import concourse.bass as bass
    import concourse.mybir as mybir
    from concourse.tile import TileContext
    from contextlib import ExitStack

    _patch_drain_split()

    f32 = mybir.dt.float32
    f32r = mybir.dt.float32r
    i32 = mybir.dt.int32
    f16 = mybir.dt.float16
    i16 = mybir.dt.int16
    wdt = f16 if HALF else f32  # dtype of the dequantized-W / x matmul path

    nc = bass.Bass()
    wq = nc.declare_dram_parameter("wq", [OSH, IN], i32, isOutput=False)
    ws = nc.declare_dram_parameter("ws", [OSH, NB], f32, isOutput=False)
    # xt carries one extra 128-row k-tile: row 3072 is all-ones, rest zero —
    # used to add the bias through the regular K=128 matmul accumulation.
    xt = nc.declare_dram_parameter("xt", [IN + 128, B], wdt, isOutput=False)
    bq = nc.declare_dram_parameter("bq", [1, OSH], i32, isOutput=False)
    bs = nc.declare_dram_parameter("bs", [1, OSH // BLOCK], f32, isOutput=False)
    ident = nc.declare_dram_parameter("ident", [128, 128], wdt, isOutput=False)
    y = nc.declare_dram_parameter("y", [B, OSH], f32, isOutput=True)

    with TileContext(nc) as tc, ExitStack() as ctx:
        const = ctx.enter_context(tc.tile_pool(name="const", bufs=1))
        wq_pool = ctx.enter_context(tc.tile_pool(name="wq", bufs=3))
        wp_pool = ctx.enter_context(tc.tile_pool(name="wp", bufs=5))
        wpt_pool = ctx.enter_context(tc.tile_pool(name="wpt", bufs=4))
        wptb_pool = ctx.enter_context(tc.tile_pool(name="wptb", bufs=2))
        ysb_pool = ctx.enter_context(tc.tile_pool(name="ysb", bufs=1))
        pt_pool = ctx.enter_context(tc.tile_pool(name="pt", bufs=4, space="PSUM"))  # [128,1024] fp16 = 1 bank each
        py_pool = ctx.enter_context(tc.tile_pool(name="py", bufs=2, space="PSUM"))
        scrap_pool = ctx.enter_context(tc.tile_pool(name="scrap", bufs=1, space="PSUM"))

        # --- constants / small inputs ---
        s_all = const.tile([128, OT * NB], f32)
        nc.sync.dma_start(
            s_all[:].rearrange("p (t k) -> p t k", t=OT),
            ws[:, :].rearrange("(t p) k -> p t k", p=128),
        )
        xt_sb = const.tile([128, (KT + 1) * B], wdt)
        nc.sync.dma_start(
            xt_sb[:].rearrange("p (n b) -> p n b", n=KT + 1),
            xt[:, :].rearrange("(n p) b -> p n b", p=128),
        )
        id_sb = const.tile([128, 128], wdt)
        nc.sync.dma_start(id_sb[:], ident[:, :])
        bq_sb = const.tile([1, OSH], i32)
        nc.sync.dma_start(bq_sb[:], bq[:, :])
        bs_sb = const.tile([1, OSH // BLOCK], f32)
        nc.sync.dma_start(bs_sb[:], bs[:, :])
        # Wait-absorber scratch: the TensorScalarPtr(STT) ISA struct carries at
        # most ONE sync wait (walrus "Too many sync wait commands").  Before
        # each STT we touch its input/output tiles with cheap DVE ops so the
        # DMA-completion / slot-release waits attach to those instead.
        scr = const.tile([1, 64], f32)

        # bias dequant (single partition, 1536 elems — off critical path)
        bias_sb = const.tile([1, OSH], f32)
        nc.vector.tensor_copy(scr[0:1, 0:1], bq_sb[0:1, 0:1])
        nc.vector.tensor_copy(scr[0:1, 1:2], bs_sb[0:1, 0:1])
        nc.vector.tensor_copy(scr[0:1, 3:4], s_all[0:1, 0:1])
        nc.vector.scalar_tensor_tensor(
            bias_sb[:].rearrange("o (k j) -> o k j", j=BLOCK),
            bq_sb[:].rearrange("o (k j) -> o k j", j=BLOCK),
            128.0,
            bs_sb[:].unsqueeze(2).broadcast_to([1, OSH // BLOCK, BLOCK]),
            mybir.AluOpType.subtract,
            mybir.AluOpType.mult,
        )

        y_sb = ysb_pool.tile([B, OSH], f32)

        # PE wait-absorbers: the matmul LW ISA struct also carries at most one
        # sync wait.  Touch each constant input with a K=128 M=1 N=1 matmul so
        # the one-time DMA waits are spread over separate PE instructions;
        # steady-state matmul waits then elide via Tile's vector clock.
        scrap = scrap_pool.tile([1, 4], f32)
        for i, src in enumerate((id_sb, xt_sb)):
            nc.tensor.matmul(
                scrap[0:1, i : i + 1], src[:, 0:1], src[:, 0:1],
                start=True, stop=True,
            )

        for g in range(NG):
            # two o-tiles per wq DMA (3 MB transfers ~ higher HBM efficiency).
            # SWDGE casts int32 -> int16 in flight (codes are 0..255): halves
            # the SBUF footprint and gives the DVE dequant a 16-bit input for
            # its 2x packed mode.
            wq_pair = []
            for h in range(GRP // 2):
                t0 = GRP * g + 2 * h
                wq_t = wq_pool.tile([128, 2 * IN], i16)
                nc.gpsimd.dma_start(
                    wq_t[:].rearrange("p (t f) -> p t f", t=2),
                    wq[128 * t0 : 128 * (t0 + 2), :].rearrange(
                        "(t p) f -> p t f", p=128
                    ),
                )
                wq_pair.append(wq_t)
            wps = []
            for a in range(GRP):
                t = GRP * g + a
                wq_t = wq_pair[a // 2][:, IN * (a % 2) : IN * (a % 2 + 1)]
                wp_t = wp_pool.tile([128, IN], wdt)
                nc.vector.tensor_copy(scr[0:1, 4 + t : 5 + t], wq_t[0:1, 0:1])
                nc.vector.memset(wp_t[0:1, 0:1], 0.0)
                nc.vector.scalar_tensor_tensor(
                    wp_t[:].rearrange("p (k j) -> p k j", j=BLOCK),
                    wq_t[:].rearrange("p (k j) -> p k j", j=BLOCK),
                    128.0,
                    s_all[:, t * NB : (t + 1) * NB]
                    .unsqueeze(2)
                    .broadcast_to([128, NB, BLOCK]),
                    mybir.AluOpType.subtract,
                    mybir.AluOpType.mult,
                )
                wps.append(wp_t)

            # bias row tile: row 0 = bias chunk, rows 1..127 = 0; contracted
            # against the ones/zeros k-tile of xt (DVE: strictly in-order)
            wpt_x = wptb_pool.tile([128, 128 * GRP], wdt)
            nc.vector.memset(wpt_x[:], 0.0)
            nc.vector.tensor_copy(
                wpt_x[0:1, :], bias_sb[0:1, 512 * g : 512 * (g + 1)]
            )

            py = py_pool.tile([B, 128 * GRP], f32)
            # process contraction tiles in pairs: one full-bank [128, 1024]
            # fp16 psum tile takes 8 transposes and is evacuated in a single
            # ACT copy (halves the per-op PSUM-read overhead)
            for jp in range(KT // 2):
                pt = pt_pool.tile([128, 256 * GRP], wdt)
                for half in range(2):
                    j = 2 * jp + half
                    for a in range(GRP):
                        nc.tensor.transpose(
                            pt[:, 512 * half + 128 * a : 512 * half + 128 * (a + 1)],
                            wps[a][:, 128 * j : 128 * (j + 1)],
                            id_sb[:],
                        )
                wpt = wpt_pool.tile([128, 256 * GRP], wdt)
                nc.scalar.copy(wpt[:], pt[:])
                for half in range(2):
                    j = 2 * jp + half
                    nc.tensor.matmul(
                        py[:],
                        xt_sb[:, B * j : B * (j + 1)],
                        wpt[:, 512 * half : 512 * (half + 1)],
                        start=(j == 0),
                        stop=False,
                    )
            # += bias via the ones/zeros k-tile (K=128 like every other matmul)
            nc.tensor.matmul(
                py[:],
                xt_sb[:, B * KT : B * (KT + 1)],
                wpt_x[:],
                start=False,
                stop=True,
            )
            nc.scalar.copy(y_sb[:, 512 * g : 512 * (g + 1)], py[:])

        nc.sync.dma_start(y[:, :], y_sb[:])

    _strip_self_waits(nc, mybir)
    return nc


# NOTE: Pool (GPSIMD) is deliberately absent — it is 8 parallel Q7 cores, so
# same-engine ordering does NOT hold there and its self-waits are load-bearing.
_ENGINE_SEM_PREFIX = {
    "PE": "PE_",
    "DVE": "DVE_",
    "Activation": "Activation_",
    "SP": "SP_",
}


def _strip_self_waits(nc, mybir):
    """Several TRN2 ISA instruction structs encode at most ONE sync wait
    (walrus: "Too many sync wait commands").  Two classes of Tile-emitted
    waits are redundant and safe to drop from instructions carrying >=2:

    1. Self-engine waits: an engine completes its own instructions in order.
    2. DMAHW waits on the wq streaming loads: the slot's previous DMA was
       fully consumed by the DVE dequant before the slot-release (DVE) wait
       tick, so the DVE wait transitively covers the DMA-WAW ordering (Tile's
       per-proc vector clock does not track transitivity).
    """
    fn = nc.m.functions[0]
    # (engine, sem) -> highest value this engine has already waited for.  An
    # engine's instruction stream executes in order through the linear block
    # chain, so any later wait with value <= that is redundant.
    observed: dict = {}
    for b in fn.blocks:
        for inst in b.instructions:
            si = inst.sync_info
            if si is None or not si.on_wait:
                continue
            eng = str(inst.engine)
            if len(si.on_wait) < 2:
                for w in si.on_wait:
                    k = (eng, w.ant_name)
                    observed[k] = max(observed.get(k, 0), w.wait_value)
                continue
            keep = [
                w
                for w in si.on_wait
                if observed.get((eng, w.ant_name), 0) < w.wait_value
            ]
            pref = _ENGINE_SEM_PREFIX.get(str(inst.engine).split(".")[-1])
            if pref is not None:
                keep = [w for w in keep if not w.ant_name.startswith(pref)]
            if len(keep) >= 2 and type(inst).__name__ == "InstDMACopy":
                # In this kernel every DMA's cross-lane (DMAHW) waits guard
                # slot reuse whose previous reader/writer chain ends in the
                # compute-engine wait Tile also emitted — transitively
                # covered, so keep only the engine-sem wait.
                if any(
                    not w.ant_name.startswith(("DMAHW", "DMASW")) for w in keep
                ):
                    keep = [
                        w
                        for w in keep
                        if not w.ant_name.startswith(("DMAHW", "DMASW"))
                    ]
            for w in keep:
                k = (eng, w.ant_name)
                observed[k] = max(observed.get(k, 0), w.wait_value)
            if len(keep) != len(si.on_wait):
                inst.sync_info = mybir.SyncInfo(
                    on_wait=keep, on_update=si.on_update
                )


def _get_nc():
    if "nc" not in _CACHE:
        _CACHE["nc"] = _build_nc()
    return _CACHE["nc"]


def _make_in_maps(x, w_q, w_scales, b_q, b_scales):
    xdt = np.float16 if HALF else np.float32
    x2 = np.ascontiguousarray(x.reshape(B, IN), dtype=np.float32)
    xt = np.zeros((IN + 128, B), dtype=xdt)               # [3200, 64]
    xt[:IN] = x2.T.astype(xdt)
    xt[IN] = 1.0                                          # bias ones-row
    wq_full = np.ascontiguousarray(w_q.reshape(OUT, IN))  # int32 codes
    ws_full = np.ascontiguousarray(w_scales)              # [12288, 96]
    bq_full = np.ascontiguousarray(b_q.reshape(OUT))      # int32 codes
    bs_full = np.ascontiguousarray(b_scales)              # [384]
    ident = np.eye(128, dtype=xdt)

    in_maps = []
    for c in range(NCORES):
        o0, o1 = c * OSH, (c + 1) * OSH
        in_maps.append(
            {
                "wq": np.ascontiguousarray(wq_full[o0:o1]),
                "ws": np.ascontiguousarray(ws_full[o0:o1]),
                "xt": xt,
                "bq": np.ascontiguousarray(bq_full[o0:o1]).reshape(1, OSH),
                "bs": np.ascontiguousarray(
                    bs_full[o0 // BLOCK : o1 // BLOCK]
                ).reshape(1, OSH // BLOCK),
                "ident": ident,
            }
        )
    return in_maps


def run_shards(x, w_q, w_scales, b_q, b_scales, trace=False):
    """Run the SPMD kernel; returns (y_full, BassKernelResults)."""
    from concourse.bass_utils import run_bass_kernel_spmd

    nc = _get_nc()
    in_maps = _make_in_maps(x, w_q, w_scales, b_q, b_scales)
    res = run_bass_kernel_spmd(
        nc, in_maps, core_ids=list(range(NCORES)), trace=trace
    )
    shards = [np.asarray(res.results[c]["y"]) for c in range(NCORES)]
    y = np.concatenate(shards, axis=1).reshape(B, 1, OUT)
    return y, res


def kernel(**inputs):
    y, _ = run_shards(
        inputs["x"],
        inputs["w_q"],
        inputs["w_scales"],
        inputs["b_q"],
        inputs["b_scales"],
        trace=False,
    )
    return y.astype(np.float32)


# revision 62
# speedup vs baseline: 1.1411x; 1.1411x over previous
"""DequantingLinear Trainium2 kernel.

y = x @ W^T + b where W = (w_q - 128) * w_scales (GGML Q8_0-style, block=32),
b = (b_q - 128) * b_scales.

Sharding: column-parallel over out_features across 8 cores (1536 rows of W
per core).  Each core:
  1. DMAs its w_q shard [1536, 3072] int32 tile-by-tile (contiguous, HBM-bound)
  2. Dequantizes on DVE with one fused scalar_tensor_tensor:
         wp = (w_q - 128) * scales   (scales broadcast along free dim, step-0 AP)
  3. Transposes wp 128x128 tiles on the PE (is_transpose matmul) -> PSUM,
     evacuates to SBUF on ACT  (the matmul needs W^T: contraction dim must be
     on partitions for both operands)
  4. Accumulates y[64, 512] = sum_k xT_k^T @ wpT_k in PSUM (24 fp32 matmuls,
     N=512) + one K=1 matmul against ones to add the (device-dequantized) bias
  5. DMAs y shard [64, 1536] out; host concatenates shards.

x is transposed on the host (it is the tiny replicated activation, 768 KB);
all heavy tensors (w_q, w_scales) stream through the device untouched.
"""

import sys

import numpy as np

for _p in ("/opt/trn_rl_repo", "/root/.axon_site/_ro/trn_rl_repo"):
    if _p not in sys.path:
        sys.path.append(_p)

B = 64          # batch (x is [64, 1, 3072])
IN = 3072       # in_features
OUT = 12288     # out_features
BLOCK = 32      # quant block
NB = IN // BLOCK            # 96 blocks per row
NCORES = 8
OSH = OUT // NCORES         # 1536 out features per core
OT = OSH // 128             # 12 o-tiles of 128 rows per core
GRP = 4                     # o-tiles per matmul group (N = 512)
NG = OT // GRP              # 3 groups
KT = IN // 128              # 24 contraction tiles

_CACHE: dict = {}

# PE dtype modes (empirically tuned; see notes):
#  - fp32 matmul = 2 half-speed instruction pairs (4 cycles/row)
#  - float32r = single-pass (1 cycle/row at N>=256); transposes 1.5 vs 2.0
F32R_TR = False  # float32r needs producers rounded (bf16-class precision)
F32R_MM = False
TRMODE_MM = False  # is_transpose with a non-identity rhs does NOT matmul (NaN)
# Half-precision W/x matmul path: dequant stays fp32 on DVE, W is rounded once
# to fp16; PE transposes and matmuls run at 1 cycle/row (vs 4 for fp32) with
# fp32 PSUM accumulation.  fp32 matmul on TRN2 costs 2 half-rate instruction
# passes, so the ~56us/core memory roofline is only reachable this way.
# fp16 (11-bit mantissa) gives ~2.4e-4 relative error vs bf16's ~2e-3 at the
# same PE speed; the value ranges (|W|<=2.6, |x|<6) are far from fp16 limits
# and accumulation is fp32 in PSUM.
HALF = True


def _patch_drain_split():
    """The TRN2 ISA gives every instruction exactly ONE inline wait slot;
    Tile's kernel-tail drain asks for the whole global clock (~11 sems) on a
    single instruction, which walrus sometimes refuses ("Too many sync wait
    commands").  Pre-spread those waits across one SP nop per semaphore; the
    drain's own waits then elide via the SP engine clock."""
    from concourse import tile as tile_mod

    if getattr(tile_mod.TileContext, "_drain_split_patched", False):
        return
    from concourse.vector_clock import ScopedClock, VectorClock

    orig = tile_mod.TileContext._drain_and_barrier

    def patched(self, tick_clock, wait_clock):
        gvc = tick_clock.global_clock
        n = len(gvc)
        for p in range(n):
            t = gvc[p]
            if t <= 0:
                continue
            vc = VectorClock([0] * n)
            vc.require_at_least(p, t)
            nop = self.nc.sync.nop(hint="drain_wait_split", nofuse=True)
            wait_clock.add_sem_waits(nop.ins, ScopedClock({None: vc}))
        return orig(self, tick_clock, wait_clock)

    tile_mod.TileContext._drain_and_barrier = patched
    tile_mod.TileContext._drain_split_patched = True


def _build_nc():
    import concourse.bass as bass
    import concourse.mybir as mybir
    from concourse.tile import TileContext
    from contextlib import ExitStack

    _patch_drain_split()

    f32 = mybir.dt.float32
    f32r = mybir.dt.float32r
    i32 = mybir.dt.int32
    f16 = mybir.dt.float16
    i16 = mybir.dt.int16
    wdt = f16 if HALF else f32  # dtype of the dequantized-W / x matmul path

    nc = bass.Bass()
    wq = nc.declare_dram_parameter("wq", [OSH, IN], i32, isOutput=False)
    ws = nc.declare_dram_parameter("ws", [OSH, NB], f32, isOutput=False)
    # xt carries one extra 128-row k-tile: row 3072 is all-ones, rest zero —
    # used to add the bias through the regular K=128 matmul accumulation.
    xt = nc.declare_dram_parameter("xt", [IN + 128, B], wdt, isOutput=False)
    bq = nc.declare_dram_parameter("bq", [1, OSH], i32, isOutput=False)
    bs = nc.declare_dram_parameter("bs", [1, OSH // BLOCK], f32, isOutput=False)
    ident = nc.declare_dram_parameter("ident", [128, 128], wdt, isOutput=False)
    y = nc.declare_dram_parameter("y", [B, OSH], f32, isOutput=True)

    with TileContext(nc) as tc, ExitStack() as ctx:
        const = ctx.enter_context(tc.tile_pool(name="const", bufs=1))
        wq_pool = ctx.enter_context(tc.tile_pool(name="wq", bufs=3))
        wp_pool = ctx.enter_context(tc.tile_pool(name="wp", bufs=5))
        wpt_pool = ctx.enter_context(tc.tile_pool(name="wpt", bufs=4))
        wptb_pool = ctx.enter_context(tc.tile_pool(name="wptb", bufs=2))
        ysb_pool = ctx.enter_context(tc.tile_pool(name="ysb", bufs=1))
        pt_pool = ctx.enter_context(tc.tile_pool(name="pt", bufs=4, space="PSUM"))  # [128,1024] fp16 = 1 bank each
        py_pool = ctx.enter_context(tc.tile_pool(name="py", bufs=2, space="PSUM"))
        scrap_pool = ctx.enter_context(tc.tile_pool(name="scrap", bufs=1, space="PSUM"))

        # --- constants / small inputs ---
        s_all = const.tile([128, OT * NB], f32)
        nc.sync.dma_start(
            s_all[:].rearrange("p (t k) -> p t k", t=OT),
            ws[:, :].rearrange("(t p) k -> p t k", p=128),
        )
        xt_sb = const.tile([128, (KT + 1) * B], wdt)
        nc.sync.dma_start(
            xt_sb[:].rearrange("p (n b) -> p n b", n=KT + 1),
            xt[:, :].rearrange("(n p) b -> p n b", p=128),
        )
        id_sb = const.tile([128, 128], wdt)
        nc.sync.dma_start(id_sb[:], ident[:, :])
        bq_sb = const.tile([1, OSH], i32)
        nc.sync.dma_start(bq_sb[:], bq[:, :])
        bs_sb = const.tile([1, OSH // BLOCK], f32)
        nc.sync.dma_start(bs_sb[:], bs[:, :])
        # Wait-absorber scratch: the TensorScalarPtr(STT) ISA struct carries at
        # most ONE sync wait (walrus "Too many sync wait commands").  Before
        # each STT we touch its input/output tiles with cheap DVE ops so the
        # DMA-completion / slot-release waits attach to those instead.
        scr = const.tile([1, 64], f32)

        # bias dequant (single partition, 1536 elems — off critical path)
        bias_sb = const.tile([1, OSH], f32)
        nc.vector.tensor_copy(scr[0:1, 0:1], bq_sb[0:1, 0:1])
        nc.vector.tensor_copy(scr[0:1, 1:2], bs_sb[0:1, 0:1])
        nc.vector.tensor_copy(scr[0:1, 3:4], s_all[0:1, 0:1])
        nc.vector.scalar_tensor_tensor(
            bias_sb[:].rearrange("o (k j) -> o k j", j=BLOCK),
            bq_sb[:].rearrange("o (k j) -> o k j", j=BLOCK),
            128.0,
            bs_sb[:].unsqueeze(2).broadcast_to([1, OSH // BLOCK, BLOCK]),
            mybir.AluOpType.subtract,
            mybir.AluOpType.mult,
        )

        y_sb = ysb_pool.tile([B, OSH], f32)

        # PE wait-absorbers: the matmul LW ISA struct also carries at most one
        # sync wait.  Touch each constant input with a K=128 M=1 N=1 matmul so
        # the one-time DMA waits are spread over separate PE instructions;
        # steady-state matmul waits then elide via Tile's vector clock.
        scrap = scrap_pool.tile([1, 4], f32)
        for i, src in enumerate((id_sb, xt_sb)):
            nc.tensor.matmul(
                scrap[0:1, i : i + 1], src[:, 0:1], src[:, 0:1],
                start=True, stop=True,
            )

        for g in range(NG):
            # two o-tiles per wq DMA (3 MB transfers ~ higher HBM efficiency;
            # each dma_start also pays an ~1-2us completion-receipt tail, so
            # fewer/bigger transfers win)
            wq_pair = []
            for h in range(GRP // 2):
                t0 = GRP * g + 2 * h
                wq_t = wq_pool.tile([128, 2 * IN], i32)
                nc.sync.dma_start(
                    wq_t[:].rearrange("p (t f) -> p t f", t=2),
                    wq[128 * t0 : 128 * (t0 + 2), :].rearrange(
                        "(t p) f -> p t f", p=128
                    ),
                )
                wq_pair.append(wq_t)
            wps = []
            for a in range(GRP):
                t = GRP * g + a
                wq_t = wq_pair[a // 2][:, IN * (a % 2) : IN * (a % 2 + 1)]
                wp_t = wp_pool.tile([128, IN], wdt)
                nc.vector.tensor_copy(scr[0:1, 4 + t : 5 + t], wq_t[0:1, 0:1])
                nc.vector.memset(wp_t[0:1, 0:1], 0.0)
                nc.vector.scalar_tensor_tensor(
                    wp_t[:].rearrange("p (k j) -> p k j", j=BLOCK),
                    wq_t[:].rearrange("p (k j) -> p k j", j=BLOCK),
                    128.0,
                    s_all[:, t * NB : (t + 1) * NB]
                    .unsqueeze(2)
                    .broadcast_to([128, NB, BLOCK]),
                    mybir.AluOpType.subtract,
                    mybir.AluOpType.mult,
                )
                wps.append(wp_t)

            # bias row tile: row 0 = bias chunk, rows 1..127 = 0; contracted
            # against the ones/zeros k-tile of xt (DVE: strictly in-order)
            wpt_x = wptb_pool.tile([128, 128 * GRP], wdt)
            nc.vector.memset(wpt_x[:], 0.0)
            nc.vector.tensor_copy(
                wpt_x[0:1, :], bias_sb[0:1, 512 * g : 512 * (g + 1)]
            )

            py = py_pool.tile([B, 128 * GRP], f32)
            # process contraction tiles in pairs: one full-bank [128, 1024]
            # fp16 psum tile takes 8 transposes and is evacuated in a single
            # ACT copy (halves the per-op PSUM-read overhead)
            for jp in range(KT // 2):
                pt = pt_pool.tile([128, 256 * GRP], wdt)
                for half in range(2):
                    j = 2 * jp + half
                    for a in range(GRP):
                        nc.tensor.transpose(
                            pt[:, 512 * half + 128 * a : 512 * half + 128 * (a + 1)],
                            wps[a][:, 128 * j : 128 * (j + 1)],
                            id_sb[:],
                        )
                wpt = wpt_pool.tile([128, 256 * GRP], wdt)
                nc.scalar.copy(wpt[:], pt[:])
                for half in range(2):
                    j = 2 * jp + half
                    nc.tensor.matmul(
                        py[:],
                        xt_sb[:, B * j : B * (j + 1)],
                        wpt[:, 512 * half : 512 * (half + 1)],
                        start=(j == 0),
                        stop=False,
                    )
            # += bias via the ones/zeros k-tile (K=128 like every other matmul)
            nc.tensor.matmul(
                py[:],
                xt_sb[:, B * KT : B * (KT + 1)],
                wpt_x[:],
                start=False,
                stop=True,
            )
            nc.scalar.copy(y_sb[:, 512 * g : 512 * (g + 1)], py[:])

        nc.sync.dma_start(y[:, :], y_sb[:])

    _strip_self_waits(nc, mybir)
    return nc


# NOTE: Pool (GPSIMD) is deliberately absent — it is 8 parallel Q7 cores, so
# same-engine ordering does NOT hold there and its self-waits are load-bearing.
_ENGINE_SEM_PREFIX = {
    "PE": "PE_",
    "DVE": "DVE_",
    "Activation": "Activation_",
    "SP": "SP_",
}


def _strip_self_waits(nc, mybir):
    """Several TRN2 ISA instruction structs encode at most ONE sync wait
    (walrus: "Too many sync wait commands").  Two classes of Tile-emitted
    waits are redundant and safe to drop from instructions carrying >=2:

    1. Self-engine waits: an engine completes its own instructions in order.
    2. DMAHW waits on the wq streaming loads: the slot's previous DMA was
       fully consumed by the DVE dequant before the slot-release (DVE) wait
       tick, so the DVE wait transitively covers the DMA-WAW ordering (Tile's
       per-proc vector clock does not track transitivity).
    """
    fn = nc.m.functions[0]
    # (engine, sem) -> highest value this engine has already waited for.  An
    # engine's instruction stream executes in order through the linear block
    # chain, so any later wait with value <= that is redundant.
    observed: dict = {}
    for b in fn.blocks:
        for inst in b.instructions:
            si = inst.sync_info
            if si is None or not si.on_wait:
                continue
            eng = str(inst.engine)
            if len(si.on_wait) < 2:
                for w in si.on_wait:
                    k = (eng, w.ant_name)
                    observed[k] = max(observed.get(k, 0), w.wait_value)
                continue
            keep = [
                w
                for w in si.on_wait
                if observed.get((eng, w.ant_name), 0) < w.wait_value
            ]
            pref = _ENGINE_SEM_PREFIX.get(str(inst.engine).split(".")[-1])
            if pref is not None:
                keep = [w for w in keep if not w.ant_name.startswith(pref)]
            if len(keep) >= 2 and type(inst).__name__ == "InstDMACopy":
                # In this kernel every DMA's cross-lane (DMAHW) waits guard
                # slot reuse whose previous reader/writer chain ends in the
                # compute-engine wait Tile also emitted — transitively
                # covered, so keep only the engine-sem wait.
                if any(
                    not w.ant_name.startswith(("DMAHW", "DMASW")) for w in keep
                ):
                    keep = [
                        w
                        for w in keep
                        if not w.ant_name.startswith(("DMAHW", "DMASW"))
                    ]
            for w in keep:
                k = (eng, w.ant_name)
                observed[k] = max(observed.get(k, 0), w.wait_value)
            if len(keep) != len(si.on_wait):
                inst.sync_info = mybir.SyncInfo(
                    on_wait=keep, on_update=si.on_update
                )


def _get_nc():
    if "nc" not in _CACHE:
        _CACHE["nc"] = _build_nc()
    return _CACHE["nc"]


def _make_in_maps(x, w_q, w_scales, b_q, b_scales):
    xdt = np.float16 if HALF else np.float32
    x2 = np.ascontiguousarray(x.reshape(B, IN), dtype=np.float32)
    xt = np.zeros((IN + 128, B), dtype=xdt)               # [3200, 64]
    xt[:IN] = x2.T.astype(xdt)
    xt[IN] = 1.0                                          # bias ones-row
    wq_full = np.ascontiguousarray(w_q.reshape(OUT, IN))  # int32 codes
    ws_full = np.ascontiguousarray(w_scales)              # [12288, 96]
    bq_full = np.ascontiguousarray(b_q.reshape(OUT))      # int32 codes
    bs_full = np.ascontiguousarray(b_scales)              # [384]
    ident = np.eye(128, dtype=xdt)

    in_maps = []
    for c in range(NCORES):
        o0, o1 = c * OSH, (c + 1) * OSH
        in_maps.append(
            {
                "wq": np.ascontiguousarray(wq_full[o0:o1]),
                "ws": np.ascontiguousarray(ws_full[o0:o1]),
                "xt": xt,
                "bq": np.ascontiguousarray(bq_full[o0:o1]).reshape(1, OSH),
                "bs": np.ascontiguousarray(
                    bs_full[o0 // BLOCK : o1 // BLOCK]
                ).reshape(1, OSH // BLOCK),
                "ident": ident,
            }
        )
    return in_maps


def run_shards(x, w_q, w_scales, b_q, b_scales, trace=False):
    """Run the SPMD kernel; returns (y_full, BassKernelResults)."""
    from concourse.bass_utils import run_bass_kernel_spmd

    nc = _get_nc()
    in_maps = _make_in_maps(x, w_q, w_scales, b_q, b_scales)
    res = run_bass_kernel_spmd(
        nc, in_maps, core_ids=list(range(NCORES)), trace=trace
    )
    shards = [np.asarray(res.results[c]["y"]) for c in range(NCORES)]
    y = np.concatenate(shards, axis=1).reshape(B, 1, OUT)
    return y, res


def kernel(**inputs):
    y, _ = run_shards(
        inputs["x"],
        inputs["w_q"],
        inputs["w_scales"],
        inputs["b_q"],
        inputs["b_scales"],
        trace=False,
    )
    return y.astype(np.float32)


# revision 76
# speedup vs baseline: 1.3168x; 1.1540x over previous
"""DequantingLinear Trainium2 kernel (~88 us HW, memory-roofline bound).

y = x @ W^T + b where W = (w_q - 128) * w_scales (GGML Q8_0-style, block=32),
b = (b_q - 128) * b_scales.

Sharding: column-parallel over out_features across 8 cores (1536 rows of W
per core, 18.9 MB of int32 codes each — the HBM-bound stream).  Per core,
pipelined per 128-row o-tile so every tile's work chases its own DMA:
  1. w_q shard streams in contiguously (1.5/3 MB HWDGE transfers)
  2. DVE dequantizes with ONE fused scalar_tensor_tensor per tile:
         wp = (w_q - 128) * scales -> fp16
     (scales broadcast along the free dim with a step-0 AP; fp16 output is
     what lets the PE run 1 cycle/row — fp32 matmul costs 4 — while keeping
     ~3e-4 relative error, vs ~2e-3 for bf16)
  3. PE transposes wp 128x128 tiles (is_transpose matmul vs identity) into
     full-bank [128,1024] fp16 PSUM tiles; ACT evacuates each in one copy
     (the matmul needs W^T: contraction must be on partitions for both
     operands, and no AP can swap the partition axis)
  4. PE accumulates y[64, 128] = sum_k xT_k^T @ wpT_k in fp32 PSUM
     (24 fp16 matmuls) + one extra k-tile of ones/zeros rows in xt that
     contracts against a bias row tile -> adds the device-dequantized bias
  5. y shard [64, 1536] DMAs out; the host concatenates the 8 shards.

x is transposed/padded on the host (tiny replicated activation, <1 MB); all
heavy tensors (w_q, w_scales, b_q) stream through the device untouched.

Two TRN2 toolchain quirks are handled explicitly (see _strip_self_waits and
_patch_drain_split): every ISA instruction encodes at most ONE semaphore
wait, and walrus refuses multi-wait encodings for several instruction
structs ("Too many sync wait commands").  Cheap same-engine "absorber" ops
take the DMA/slot-release waits up front, a post-pass drops provably
redundant waits (self-engine ordering; DMA-lane waits transitively covered
by consumer-engine waits), and the kernel-tail drain's global-clock waits
are pre-spread across SP nops.
"""

import sys

import numpy as np

for _p in ("/opt/trn_rl_repo", "/root/.axon_site/_ro/trn_rl_repo"):
    if _p not in sys.path:
        sys.path.append(_p)

B = 64          # batch (x is [64, 1, 3072])
IN = 3072       # in_features
OUT = 12288     # out_features
BLOCK = 32      # quant block
NB = IN // BLOCK            # 96 blocks per row
NCORES = 8
OSH = OUT // NCORES         # 1536 out features per core
OT = OSH // 128             # 12 o-tiles of 128 rows per core
GRP = 4                     # o-tiles per matmul group (N = 512)
NG = OT // GRP              # 3 groups
KT = IN // 128              # 24 contraction tiles

_CACHE: dict = {}

# Half-precision W/x matmul path: dequant stays fp32 on DVE, W is rounded once
# to fp16; PE transposes and matmuls run at 1 cycle/row (vs 4 for fp32) with
# fp32 PSUM accumulation.  fp32 matmul on TRN2 costs 2 half-rate instruction
# passes, so the ~56us/core memory roofline is only reachable this way.
# fp16 (11-bit mantissa) gives ~2.4e-4 relative error vs bf16's ~2e-3 at the
# same PE speed; the value ranges (|W|<=2.6, |x|<6) are far from fp16 limits
# and accumulation is fp32 in PSUM.
HALF = True


def _patch_drain_split():
    """The TRN2 ISA gives every instruction exactly ONE inline wait slot;
    Tile's kernel-tail drain asks for the whole global clock (~11 sems) on a
    single instruction, which walrus sometimes refuses ("Too many sync wait
    commands").  Pre-spread those waits across one SP nop per semaphore; the
    drain's own waits then elide via the SP engine clock."""
    from concourse import tile as tile_mod

    if getattr(tile_mod.TileContext, "_drain_split_patched", False):
        return
    from concourse.vector_clock import ScopedClock, VectorClock

    orig = tile_mod.TileContext._drain_and_barrier

    def patched(self, tick_clock, wait_clock):
        gvc = tick_clock.global_clock
        n = len(gvc)
        for p in range(n):
            t = gvc[p]
            if t <= 0:
                continue
            vc = VectorClock([0] * n)
            vc.require_at_least(p, t)
            nop = self.nc.sync.nop(hint="drain_wait_split", nofuse=True)
            wait_clock.add_sem_waits(nop.ins, ScopedClock({None: vc}))
        return orig(self, tick_clock, wait_clock)

    tile_mod.TileContext._drain_and_barrier = patched
    tile_mod.TileContext._drain_split_patched = True


def _build_nc():
    import concourse.bass as bass
    import concourse.mybir as mybir
    from concourse.tile import TileContext
    from contextlib import ExitStack

    _patch_drain_split()

    f32 = mybir.dt.float32
    i32 = mybir.dt.int32
    f16 = mybir.dt.float16
    wdt = f16 if HALF else f32  # dtype of the dequantized-W / x matmul path

    nc = bass.Bass()
    wq = nc.declare_dram_parameter("wq", [OSH, IN], i32, isOutput=False)
    ws = nc.declare_dram_parameter("ws", [OSH, NB], f32, isOutput=False)
    # xt carries one extra 128-row k-tile: row 3072 is all-ones, rest zero —
    # used to add the bias through the regular K=128 matmul accumulation.
    xt = nc.declare_dram_parameter("xt", [IN + 128, B], wdt, isOutput=False)
    bq = nc.declare_dram_parameter("bq", [1, OSH], i32, isOutput=False)
    bs = nc.declare_dram_parameter("bs", [1, OSH // BLOCK], f32, isOutput=False)
    ident = nc.declare_dram_parameter("ident", [128, 128], wdt, isOutput=False)
    y = nc.declare_dram_parameter("y", [B, OSH], f32, isOutput=True)

    with TileContext(nc) as tc, ExitStack() as ctx:
        const = ctx.enter_context(tc.tile_pool(name="const", bufs=1))
        wq_pool = ctx.enter_context(tc.tile_pool(name="wq", bufs=3))
        wq1_pool = ctx.enter_context(tc.tile_pool(name="wq1", bufs=4))
        wp_pool = ctx.enter_context(tc.tile_pool(name="wp", bufs=5))
        wpt_pool = ctx.enter_context(tc.tile_pool(name="wpt", bufs=4))
        wptb_pool = ctx.enter_context(tc.tile_pool(name="wptb", bufs=2))
        ysb_pool = ctx.enter_context(tc.tile_pool(name="ysb", bufs=1))
        pt_pool = ctx.enter_context(tc.tile_pool(name="pt", bufs=6, space="PSUM"))  # [128,1024] fp16 = 1 bank each
        py_pool = ctx.enter_context(tc.tile_pool(name="py", bufs=2, space="PSUM"))

        # --- constants / small inputs ---
        s_all = const.tile([128, OT * NB], f32)
        nc.sync.dma_start(
            s_all[:].rearrange("p (t k) -> p t k", t=OT),
            ws[:, :].rearrange("(t p) k -> p t k", p=128),
        )
        xt_sb = const.tile([128, (KT + 1) * B], wdt)
        nc.sync.dma_start(
            xt_sb[:].rearrange("p (n b) -> p n b", n=KT + 1),
            xt[:, :].rearrange("(n p) b -> p n b", p=128),
        )
        id_sb = const.tile([128, 128], wdt)
        nc.sync.dma_start(id_sb[:], ident[:, :])
        # Wait-absorber scratch: the TensorScalarPtr(STT) ISA struct carries at
        # most ONE sync wait (walrus "Too many sync wait commands").  Before
        # each STT we touch its input/output tiles with cheap DVE ops so the
        # DMA-completion / slot-release waits attach to those instead.
        scr = const.tile([1, 64], f32)
        bq_sb = const.tile([1, OSH], i32)
        nc.sync.dma_start(bq_sb[:], bq[:, :])
        bs_sb = const.tile([1, OSH // BLOCK], f32)
        nc.sync.dma_start(bs_sb[:], bs[:, :])

        # bias dequant (single partition, 1536 elems — off critical path)
        bias_sb = const.tile([1, OSH], f32)
        nc.vector.tensor_copy(scr[0:1, 0:1], bq_sb[0:1, 0:1])
        nc.vector.tensor_copy(scr[0:1, 1:2], bs_sb[0:1, 0:1])
        nc.vector.tensor_copy(scr[0:1, 3:4], s_all[0:1, 0:1])
        nc.vector.scalar_tensor_tensor(
            bias_sb[:].rearrange("o (k j) -> o k j", j=BLOCK),
            bq_sb[:].rearrange("o (k j) -> o k j", j=BLOCK),
            128.0,
            bs_sb[:].unsqueeze(2).broadcast_to([1, OSH // BLOCK, BLOCK]),
            mybir.AluOpType.subtract,
            mybir.AluOpType.mult,
        )


        y_sb = ysb_pool.tile([B, OSH], f32)

        # PE wait-absorbers: the matmul LW ISA struct also carries at most one
        # sync wait.  Touch each constant input with a K=128 M=1 N=1 matmul so
        # the one-time DMA waits are spread over separate PE instructions;
        # steady-state matmul waits then elide via Tile's vector clock.
        scrap = py_pool.tile([1, 4], f32, tag="py")
        for i, src in enumerate((id_sb, xt_sb)):
            nc.tensor.matmul(
                scrap[0:1, i : i + 1], src[:, 0:1], src[:, 0:1],
                start=True, stop=True,
            )

        # wq DMAs: two o-tiles per transfer (3 MB ~ higher HBM efficiency;
        # each dma_start also pays an ~1-2us completion-receipt tail).  All
        # downstream work is per-SINGLE-o-tile so nothing gates on a late
        # neighbour tile: each tile's transposes/evac/matmuls chase its own
        # dequant, which minimises both the pipeline ramp and the drain tail.
        # first four tiles as 1.5 MB singles so the pipeline starts as early
        # as possible; the rest as 3 MB pairs (better HBM efficiency per
        # dma_start completion-receipt tail)
        wq_first = []
        for t in range(4):
            wq_s = wq1_pool.tile([128, IN], i32)
            nc.sync.dma_start(wq_s[:], wq[128 * t : 128 * (t + 1), :])
            wq_first.append(wq_s)
        wq_pair = []
        for h in range(2, OT // 2):
            wq_t = wq_pool.tile([128, 2 * IN], i32)
            nc.sync.dma_start(
                wq_t[:].rearrange("p (t f) -> p t f", t=2),
                wq[256 * h : 256 * (h + 1), :].rearrange(
                    "(t p) f -> p t f", p=128
                ),
            )
            wq_pair.append(wq_t)

        for t in range(OT):
            if t < 4:
                wq_t = wq_first[t][:, :]
            else:
                wq_t = wq_pair[t // 2 - 2][:, IN * (t % 2) : IN * (t % 2 + 1)]
            wp_t = wp_pool.tile([128, IN], wdt)
            nc.vector.tensor_copy(scr[0:1, 4 + t : 5 + t], wq_t[0:1, 0:1])
            nc.vector.memset(wp_t[0:1, 0:1], 0.0)
            nc.vector.scalar_tensor_tensor(
                wp_t[:].rearrange("p (k j) -> p k j", j=BLOCK),
                wq_t[:].rearrange("p (k j) -> p k j", j=BLOCK),
                128.0,
                s_all[:, t * NB : (t + 1) * NB]
                .unsqueeze(2)
                .broadcast_to([128, NB, BLOCK]),
                mybir.AluOpType.subtract,
                mybir.AluOpType.mult,
            )

            # bias row tile: row 0 = bias chunk, rows 1..127 = 0; contracted
            # against the ones/zeros k-tile of xt (DVE: strictly in-order)
            wpt_x = wptb_pool.tile([128, 128], wdt)
            nc.vector.memset(wpt_x[:], 0.0)
            nc.vector.tensor_copy(
                wpt_x[0:1, :], bias_sb[0:1, 128 * t : 128 * (t + 1)]
            )

            py = py_pool.tile([B, 128], f32)
            # 8 contraction slices per full-bank [128, 1024] fp16 psum tile:
            # 8 transposes then ONE big ACT evacuation
            for jp in range(KT // 8):
                pt = pt_pool.tile([128, 1024], wdt)
                for jj in range(8):
                    j = 8 * jp + jj
                    nc.tensor.transpose(
                        pt[:, 128 * jj : 128 * (jj + 1)],
                        wp_t[:, 128 * j : 128 * (j + 1)],
                        id_sb[:],
                    )
                wpt = wpt_pool.tile([128, 1024], wdt)
                nc.scalar.copy(wpt[:], pt[:])
                for jj in range(8):
                    j = 8 * jp + jj
                    nc.tensor.matmul(
                        py[:],
                        xt_sb[:, B * j : B * (j + 1)],
                        wpt[:, 128 * jj : 128 * (jj + 1)],
                        start=(j == 0),
                        stop=False,
                    )
            # += bias via the ones/zeros k-tile (K=128 like every other matmul)
            nc.tensor.matmul(
                py[:],
                xt_sb[:, B * KT : B * (KT + 1)],
                wpt_x[:],
                start=False,
                stop=True,
            )
            nc.scalar.copy(y_sb[:, 128 * t : 128 * (t + 1)], py[:])

        nc.sync.dma_start(y[:, :], y_sb[:])

    _strip_self_waits(nc, mybir)
    return nc


# NOTE: Pool (GPSIMD) is deliberately absent — it is 8 parallel Q7 cores, so
# same-engine ordering does NOT hold there and its self-waits are load-bearing.
_ENGINE_SEM_PREFIX = {
    "PE": "PE_",
    "DVE": "DVE_",
    "Activation": "Activation_",
    "SP": "SP_",
}


def _strip_self_waits(nc, mybir):
    """Several TRN2 ISA instruction structs encode at most ONE sync wait
    (walrus: "Too many sync wait commands").  Two classes of Tile-emitted
    waits are redundant and safe to drop from instructions carrying >=2:

    1. Self-engine waits: an engine completes its own instructions in order.
    2. DMAHW waits on the wq streaming loads: the slot's previous DMA was
       fully consumed by the DVE dequant before the slot-release (DVE) wait
       tick, so the DVE wait transitively covers the DMA-WAW ordering (Tile's
       per-proc vector clock does not track transitivity).
    """
    fn = nc.m.functions[0]
    # (engine, sem) -> highest value this engine has already waited for.  An
    # engine's instruction stream executes in order through the linear block
    # chain, so any later wait with value <= that is redundant.
    observed: dict = {}
    for b in fn.blocks:
        for inst in b.instructions:
            si = inst.sync_info
            if si is None or not si.on_wait:
                continue
            eng = str(inst.engine)
            if len(si.on_wait) < 2:
                for w in si.on_wait:
                    k = (eng, w.ant_name)
                    observed[k] = max(observed.get(k, 0), w.wait_value)
                continue
            keep = [
                w
                for w in si.on_wait
                if observed.get((eng, w.ant_name), 0) < w.wait_value
            ]
            pref = _ENGINE_SEM_PREFIX.get(str(inst.engine).split(".")[-1])
            if pref is not None:
                keep = [w for w in keep if not w.ant_name.startswith(pref)]
            if len(keep) >= 2 and type(inst).__name__ == "InstDMACopy":
                # In this kernel every DMA's cross-lane (DMAHW) waits guard
                # slot reuse whose previous reader/writer chain ends in the
                # compute-engine wait Tile also emitted — transitively
                # covered, so keep only the engine-sem wait.
                if any(
                    not w.ant_name.startswith(("DMAHW", "DMASW")) for w in keep
                ):
                    keep = [
                        w
                        for w in keep
                        if not w.ant_name.startswith(("DMAHW", "DMASW"))
                    ]
            for w in keep:
                k = (eng, w.ant_name)
                observed[k] = max(observed.get(k, 0), w.wait_value)
            if len(keep) != len(si.on_wait):
                inst.sync_info = mybir.SyncInfo(
                    on_wait=keep, on_update=si.on_update
                )


def _get_nc():
    if "nc" not in _CACHE:
        _CACHE["nc"] = _build_nc()
    return _CACHE["nc"]


def _make_in_maps(x, w_q, w_scales, b_q, b_scales):
    xdt = np.float16 if HALF else np.float32
    x2 = np.ascontiguousarray(x.reshape(B, IN), dtype=np.float32)
    xt = np.zeros((IN + 128, B), dtype=xdt)               # [3200, 64]
    xt[:IN] = x2.T.astype(xdt)
    xt[IN] = 1.0                                          # bias ones-row
    wq_full = np.ascontiguousarray(w_q.reshape(OUT, IN))  # int32 codes
    ws_full = np.ascontiguousarray(w_scales)              # [12288, 96]
    bq_full = np.ascontiguousarray(b_q.reshape(OUT))      # int32 codes
    bs_full = np.ascontiguousarray(b_scales)              # [384]
    ident = np.eye(128, dtype=xdt)

    in_maps = []
    for c in range(NCORES):
        o0, o1 = c * OSH, (c + 1) * OSH
        in_maps.append(
            {
                "wq": np.ascontiguousarray(wq_full[o0:o1]),
                "ws": np.ascontiguousarray(ws_full[o0:o1]),
                "xt": xt,
                "bq": np.ascontiguousarray(bq_full[o0:o1]).reshape(1, OSH),
                "bs": np.ascontiguousarray(
                    bs_full[o0 // BLOCK : o1 // BLOCK]
                ).reshape(1, OSH // BLOCK),
                "ident": ident,
            }
        )
    return in_maps


def run_shards(x, w_q, w_scales, b_q, b_scales, trace=False):
    """Run the SPMD kernel; returns (y_full, BassKernelResults)."""
    from concourse.bass_utils import run_bass_kernel_spmd

    nc = _get_nc()
    in_maps = _make_in_maps(x, w_q, w_scales, b_q, b_scales)
    res = run_bass_kernel_spmd(
        nc, in_maps, core_ids=list(range(NCORES)), trace=trace
    )
    shards = [np.asarray(res.results[c]["y"]) for c in range(NCORES)]
    y = np.concatenate(shards, axis=1).reshape(B, 1, OUT)
    return y, res


def kernel(**inputs):
    y, _ = run_shards(
        inputs["x"],
        inputs["w_q"],
        inputs["w_scales"],
        inputs["b_q"],
        inputs["b_scales"],
        trace=False,
    )
    return y.astype(np.float32)


# revision 77
# speedup vs baseline: 1.3468x; 1.0227x over previous
"""DequantingLinear Trainium2 kernel (~88 us HW, memory-roofline bound).

y = x @ W^T + b where W = (w_q - 128) * w_scales (GGML Q8_0-style, block=32),
b = (b_q - 128) * b_scales.

Sharding: column-parallel over out_features across 8 cores (1536 rows of W
per core, 18.9 MB of int32 codes each — the HBM-bound stream).  Per core,
pipelined per 128-row o-tile so every tile's work chases its own DMA:
  1. w_q shard streams in contiguously (1.5/3 MB HWDGE transfers)
  2. DVE dequantizes with ONE fused scalar_tensor_tensor per tile:
         wp = (w_q - 128) * scales -> fp16
     (scales broadcast along the free dim with a step-0 AP; fp16 output is
     what lets the PE run 1 cycle/row — fp32 matmul costs 4 — while keeping
     ~3e-4 relative error, vs ~2e-3 for bf16)
  3. PE transposes wp 128x128 tiles (is_transpose matmul vs identity) into
     full-bank [128,1024] fp16 PSUM tiles; ACT evacuates each in one copy
     (the matmul needs W^T: contraction must be on partitions for both
     operands, and no AP can swap the partition axis)
  4. PE accumulates y[64, 128] = sum_k xT_k^T @ wpT_k in fp32 PSUM
     (24 fp16 matmuls) + one extra k-tile of ones/zeros rows in xt that
     contracts against a bias row tile -> adds the device-dequantized bias
  5. y shard [64, 1536] DMAs out; the host concatenates the 8 shards.

x is transposed/padded on the host (tiny replicated activation, <1 MB); all
heavy tensors (w_q, w_scales, b_q) stream through the device untouched.

Two TRN2 toolchain quirks are handled explicitly (see _strip_self_waits and
_patch_drain_split): every ISA instruction encodes at most ONE semaphore
wait, and walrus refuses multi-wait encodings for several instruction
structs ("Too many sync wait commands").  Cheap same-engine "absorber" ops
take the DMA/slot-release waits up front, a post-pass drops provably
redundant waits (self-engine ordering; DMA-lane waits transitively covered
by consumer-engine waits), and the kernel-tail drain's global-clock waits
are pre-spread across SP nops.
"""

import sys

import numpy as np

for _p in ("/opt/trn_rl_repo", "/root/.axon_site/_ro/trn_rl_repo"):
    if _p not in sys.path:
        sys.path.append(_p)

B = 64          # batch (x is [64, 1, 3072])
IN = 3072       # in_features
OUT = 12288     # out_features
BLOCK = 32      # quant block
NB = IN // BLOCK            # 96 blocks per row
NCORES = 8
OSH = OUT // NCORES         # 1536 out features per core
OT = OSH // 128             # 12 o-tiles of 128 rows per core
GRP = 4                     # o-tiles per matmul group (N = 512)
NG = OT // GRP              # 3 groups
KT = IN // 128              # 24 contraction tiles

_CACHE: dict = {}

# Half-precision W/x matmul path: dequant stays fp32 on DVE, W is rounded once
# to fp16; PE transposes and matmuls run at 1 cycle/row (vs 4 for fp32) with
# fp32 PSUM accumulation.  fp32 matmul on TRN2 costs 2 half-rate instruction
# passes, so the ~56us/core memory roofline is only reachable this way.
# fp16 (11-bit mantissa) gives ~2.4e-4 relative error vs bf16's ~2e-3 at the
# same PE speed; the value ranges (|W|<=2.6, |x|<6) are far from fp16 limits
# and accumulation is fp32 in PSUM.
HALF = True


def _patch_drain_split():
    """The TRN2 ISA gives every instruction exactly ONE inline wait slot;
    Tile's kernel-tail drain asks for the whole global clock (~11 sems) on a
    single instruction, which walrus sometimes refuses ("Too many sync wait
    commands").  Pre-spread those waits across one SP nop per semaphore; the
    drain's own waits then elide via the SP engine clock."""
    from concourse import tile as tile_mod

    if getattr(tile_mod.TileContext, "_drain_split_patched", False):
        return
    from concourse.vector_clock import ScopedClock, VectorClock

    orig = tile_mod.TileContext._drain_and_barrier

    def patched(self, tick_clock, wait_clock):
        gvc = tick_clock.global_clock
        n = len(gvc)
        for p in range(n):
            t = gvc[p]
            if t <= 0:
                continue
            vc = VectorClock([0] * n)
            vc.require_at_least(p, t)
            nop = self.nc.sync.nop(hint="drain_wait_split", nofuse=True)
            wait_clock.add_sem_waits(nop.ins, ScopedClock({None: vc}))
        return orig(self, tick_clock, wait_clock)

    tile_mod.TileContext._drain_and_barrier = patched
    tile_mod.TileContext._drain_split_patched = True


def _build_nc():
    import concourse.bass as bass
    import concourse.mybir as mybir
    from concourse.tile import TileContext
    from contextlib import ExitStack

    _patch_drain_split()

    f32 = mybir.dt.float32
    i32 = mybir.dt.int32
    f16 = mybir.dt.float16
    wdt = f16 if HALF else f32  # dtype of the dequantized-W / x matmul path

    nc = bass.Bass()
    wq = nc.declare_dram_parameter("wq", [OSH, IN], i32, isOutput=False)
    ws = nc.declare_dram_parameter("ws", [OSH, NB], f32, isOutput=False)
    # xt carries one extra 128-row k-tile: row 3072 is all-ones, rest zero —
    # used to add the bias through the regular K=128 matmul accumulation.
    xt = nc.declare_dram_parameter("xt", [IN + 128, B], wdt, isOutput=False)
    bq = nc.declare_dram_parameter("bq", [1, OSH], i32, isOutput=False)
    bs = nc.declare_dram_parameter("bs", [1, OSH // BLOCK], f32, isOutput=False)
    ident = nc.declare_dram_parameter("ident", [128, 128], wdt, isOutput=False)
    y = nc.declare_dram_parameter("y", [B, OSH], f32, isOutput=True)

    with TileContext(nc) as tc, ExitStack() as ctx:
        const = ctx.enter_context(tc.tile_pool(name="const", bufs=1))
        wq_pool = ctx.enter_context(tc.tile_pool(name="wq", bufs=3))
        wq1_pool = ctx.enter_context(tc.tile_pool(name="wq1", bufs=4))
        wp_pool = ctx.enter_context(tc.tile_pool(name="wp", bufs=5))
        wpt_pool = ctx.enter_context(tc.tile_pool(name="wpt", bufs=4))
        wptb_pool = ctx.enter_context(tc.tile_pool(name="wptb", bufs=2))
        ysb_pool = ctx.enter_context(tc.tile_pool(name="ysb", bufs=1))
        pt_pool = ctx.enter_context(tc.tile_pool(name="pt", bufs=6, space="PSUM"))  # [128,1024] fp16 = 1 bank each
        py_pool = ctx.enter_context(tc.tile_pool(name="py", bufs=2, space="PSUM"))

        # --- constants / small inputs ---
        s_all = const.tile([128, OT * NB], f32)
        nc.sync.dma_start(
            s_all[:].rearrange("p (t k) -> p t k", t=OT),
            ws[:, :].rearrange("(t p) k -> p t k", p=128),
        )
        xt_sb = const.tile([128, (KT + 1) * B], wdt)
        nc.sync.dma_start(
            xt_sb[:].rearrange("p (n b) -> p n b", n=KT + 1),
            xt[:, :].rearrange("(n p) b -> p n b", p=128),
        )
        id_sb = const.tile([128, 128], wdt)
        nc.sync.dma_start(id_sb[:], ident[:, :])
        # Wait-absorber scratch: the TensorScalarPtr(STT) ISA struct carries at
        # most ONE sync wait (walrus "Too many sync wait commands").  Before
        # each STT we touch its input/output tiles with cheap DVE ops so the
        # DMA-completion / slot-release waits attach to those instead.
        scr = const.tile([1, 64], f32)
        bq_sb = const.tile([1, OSH], i32)
        nc.sync.dma_start(bq_sb[:], bq[:, :])
        bs_sb = const.tile([1, OSH // BLOCK], f32)
        nc.sync.dma_start(bs_sb[:], bs[:, :])

        # bias dequant (single partition, 1536 elems — off critical path)
        bias_sb = const.tile([1, OSH], f32)
        nc.vector.tensor_copy(scr[0:1, 0:1], bq_sb[0:1, 0:1])
        nc.vector.tensor_copy(scr[0:1, 1:2], bs_sb[0:1, 0:1])
        nc.vector.tensor_copy(scr[0:1, 3:4], s_all[0:1, 0:1])
        nc.vector.scalar_tensor_tensor(
            bias_sb[:].rearrange("o (k j) -> o k j", j=BLOCK),
            bq_sb[:].rearrange("o (k j) -> o k j", j=BLOCK),
            128.0,
            bs_sb[:].unsqueeze(2).broadcast_to([1, OSH // BLOCK, BLOCK]),
            mybir.AluOpType.subtract,
            mybir.AluOpType.mult,
        )


        y_sb = ysb_pool.tile([B, OSH], f32)

        # PE wait-absorbers: the matmul LW ISA struct also carries at most one
        # sync wait.  Touch each constant input with a K=128 M=1 N=1 matmul so
        # the one-time DMA waits are spread over separate PE instructions;
        # steady-state matmul waits then elide via Tile's vector clock.
        scrap = py_pool.tile([1, 4], f32, tag="py")
        for i, src in enumerate((id_sb, xt_sb)):
            nc.tensor.matmul(
                scrap[0:1, i : i + 1], src[:, 0:1], src[:, 0:1],
                start=True, stop=True,
            )

        # wq DMAs: two o-tiles per transfer (3 MB ~ higher HBM efficiency;
        # each dma_start also pays an ~1-2us completion-receipt tail).  All
        # downstream work is per-SINGLE-o-tile so nothing gates on a late
        # neighbour tile: each tile's transposes/evac/matmuls chase its own
        # dequant, which minimises both the pipeline ramp and the drain tail.
        # first four tiles as 1.5 MB singles so the pipeline starts as early
        # as possible; the rest as 3 MB pairs (better HBM efficiency per
        # dma_start completion-receipt tail)
        wq_first = []
        for t in range(4):
            wq_s = wq1_pool.tile([128, IN], i32)
            nc.sync.dma_start(wq_s[:], wq[128 * t : 128 * (t + 1), :])
            wq_first.append(wq_s)
        wq_pair = []
        for h in range(2, OT // 2):
            wq_t = wq_pool.tile([128, 2 * IN], i32)
            nc.sync.dma_start(
                wq_t[:].rearrange("p (t f) -> p t f", t=2),
                wq[256 * h : 256 * (h + 1), :].rearrange(
                    "(t p) f -> p t f", p=128
                ),
            )
            wq_pair.append(wq_t)

        for t in range(OT):
            if t < 4:
                wq_t = wq_first[t][:, :]
            else:
                wq_t = wq_pair[t // 2 - 2][:, IN * (t % 2) : IN * (t % 2 + 1)]
            wp_t = wp_pool.tile([128, IN], wdt)
            nc.vector.tensor_copy(scr[0:1, 4 + t : 5 + t], wq_t[0:1, 0:1])
            nc.vector.memset(wp_t[0:1, 0:1], 0.0)
            # dequant in two halves: the first half's transposes start ~1.7us
            # earlier, shortening the per-tile critical path and drain tail
            for hh in range(2):
                sl = slice(hh * IN // 2, (hh + 1) * IN // 2)
                nc.vector.scalar_tensor_tensor(
                    wp_t[:, sl].rearrange("p (k j) -> p k j", j=BLOCK),
                    wq_t[:, sl].rearrange("p (k j) -> p k j", j=BLOCK),
                    128.0,
                    s_all[:, t * NB + hh * NB // 2 : t * NB + (hh + 1) * NB // 2]
                    .unsqueeze(2)
                    .broadcast_to([128, NB // 2, BLOCK]),
                    mybir.AluOpType.subtract,
                    mybir.AluOpType.mult,
                )

            # bias row tile: row 0 = bias chunk, rows 1..127 = 0; contracted
            # against the ones/zeros k-tile of xt (DVE: strictly in-order)
            wpt_x = wptb_pool.tile([128, 128], wdt)
            nc.vector.memset(wpt_x[:], 0.0)
            nc.vector.tensor_copy(
                wpt_x[0:1, :], bias_sb[0:1, 128 * t : 128 * (t + 1)]
            )

            py = py_pool.tile([B, 128], f32)
            # 8 contraction slices per full-bank [128, 1024] fp16 psum tile:
            # 8 transposes then ONE big ACT evacuation
            for jp in range(KT // 8):
                pt = pt_pool.tile([128, 1024], wdt)
                for jj in range(8):
                    j = 8 * jp + jj
                    nc.tensor.transpose(
                        pt[:, 128 * jj : 128 * (jj + 1)],
                        wp_t[:, 128 * j : 128 * (j + 1)],
                        id_sb[:],
                    )
                wpt = wpt_pool.tile([128, 1024], wdt)
                nc.scalar.copy(wpt[:], pt[:])
                for jj in range(8):
                    j = 8 * jp + jj
                    nc.tensor.matmul(
                        py[:],
                        xt_sb[:, B * j : B * (j + 1)],
                        wpt[:, 128 * jj : 128 * (jj + 1)],
                        start=(j == 0),
                        stop=False,
                    )
            # += bias via the ones/zeros k-tile (K=128 like every other matmul)
            nc.tensor.matmul(
                py[:],
                xt_sb[:, B * KT : B * (KT + 1)],
                wpt_x[:],
                start=False,
                stop=True,
            )
            nc.scalar.copy(y_sb[:, 128 * t : 128 * (t + 1)], py[:])

        nc.sync.dma_start(y[:, :], y_sb[:])

    _strip_self_waits(nc, mybir)
    return nc


# NOTE: Pool (GPSIMD) is deliberately absent — it is 8 parallel Q7 cores, so
# same-engine ordering does NOT hold there and its self-waits are load-bearing.
_ENGINE_SEM_PREFIX = {
    "PE": "PE_",
    "DVE": "DVE_",
    "Activation": "Activation_",
    "SP": "SP_",
}


def _strip_self_waits(nc, mybir):
    """Several TRN2 ISA instruction structs encode at most ONE sync wait
    (walrus: "Too many sync wait commands").  Two classes of Tile-emitted
    waits are redundant and safe to drop from instructions carrying >=2:

    1. Self-engine waits: an engine completes its own instructions in order.
    2. DMAHW waits on the wq streaming loads: the slot's previous DMA was
       fully consumed by the DVE dequant before the slot-release (DVE) wait
       tick, so the DVE wait transitively covers the DMA-WAW ordering (Tile's
       per-proc vector clock does not track transitivity).
    """
    fn = nc.m.functions[0]
    # (engine, sem) -> highest value this engine has already waited for.  An
    # engine's instruction stream executes in order through the linear block
    # chain, so any later wait with value <= that is redundant.
    observed: dict = {}
    for b in fn.blocks:
        for inst in b.instructions:
            si = inst.sync_info
            if si is None or not si.on_wait:
                continue
            eng = str(inst.engine)
            if len(si.on_wait) < 2:
                for w in si.on_wait:
                    k = (eng, w.ant_name)
                    observed[k] = max(observed.get(k, 0), w.wait_value)
                continue
            keep = [
                w
                for w in si.on_wait
                if observed.get((eng, w.ant_name), 0) < w.wait_value
            ]
            pref = _ENGINE_SEM_PREFIX.get(str(inst.engine).split(".")[-1])
            if pref is not None:
                keep = [w for w in keep if not w.ant_name.startswith(pref)]
            if len(keep) >= 2 and type(inst).__name__ == "InstDMACopy":
                # In this kernel every DMA's cross-lane (DMAHW) waits guard
                # slot reuse whose previous reader/writer chain ends in the
                # compute-engine wait Tile also emitted — transitively
                # covered, so keep only the engine-sem wait.
                if any(
                    not w.ant_name.startswith(("DMAHW", "DMASW")) for w in keep
                ):
                    keep = [
                        w
                        for w in keep
                        if not w.ant_name.startswith(("DMAHW", "DMASW"))
                    ]
            for w in keep:
                k = (eng, w.ant_name)
                observed[k] = max(observed.get(k, 0), w.wait_value)
            if len(keep) != len(si.on_wait):
                inst.sync_info = mybir.SyncInfo(
                    on_wait=keep, on_update=si.on_update
                )


def _get_nc():
    if "nc" not in _CACHE:
        _CACHE["nc"] = _build_nc()
    return _CACHE["nc"]


def _make_in_maps(x, w_q, w_scales, b_q, b_scales):
    xdt = np.float16 if HALF else np.float32
    x2 = np.ascontiguousarray(x.reshape(B, IN), dtype=np.float32)
    xt = np.zeros((IN + 128, B), dtype=xdt)               # [3200, 64]
    xt[:IN] = x2.T.astype(xdt)
    xt[IN] = 1.0                                          # bias ones-row
    wq_full = np.ascontiguousarray(w_q.reshape(OUT, IN))  # int32 codes
    ws_full = np.ascontiguousarray(w_scales)              # [12288, 96]
    bq_full = np.ascontiguousarray(b_q.reshape(OUT))      # int32 codes
    bs_full = np.ascontiguousarray(b_scales)              # [384]
    ident = np.eye(128, dtype=xdt)

    in_maps = []
    for c in range(NCORES):
        o0, o1 = c * OSH, (c + 1) * OSH
        in_maps.append(
            {
                "wq": np.ascontiguousarray(wq_full[o0:o1]),
                "ws": np.ascontiguousarray(ws_full[o0:o1]),
                "xt": xt,
                "bq": np.ascontiguousarray(bq_full[o0:o1]).reshape(1, OSH),
                "bs": np.ascontiguousarray(
                    bs_full[o0 // BLOCK : o1 // BLOCK]
                ).reshape(1, OSH // BLOCK),
                "ident": ident,
            }
        )
    return in_maps


def run_shards(x, w_q, w_scales, b_q, b_scales, trace=False):
    """Run the SPMD kernel; returns (y_full, BassKernelResults)."""
    from concourse.bass_utils import run_bass_kernel_spmd

    nc = _get_nc()
    in_maps = _make_in_maps(x, w_q, w_scales, b_q, b_scales)
    res = run_bass_kernel_spmd(
        nc, in_maps, core_ids=list(range(NCORES)), trace=trace
    )
    shards = [np.asarray(res.results[c]["y"]) for c in range(NCORES)]
    y = np.concatenate(shards, axis=1).reshape(B, 1, OUT)
    return y, res


def kernel(**inputs):
    y, _ = run_shards(
        inputs["x"],
        inputs["w_q"],
        inputs["w_scales"],
        inputs["b_q"],
        inputs["b_scales"],
        trace=False,
    )
    return y.astype(np.float32)
